# revision 8
# baseline (speedup 1.0000x reference)
"""Trainium2 Bass kernel for nn_BotUpSaliency (B=2, H=W=512, K=12, 16 steps).

Math
----
The reference integrates, for 16 Euler steps (EPS=0.01):

    y'  = y + EPS*(-y + gx + conv(gx,W) + 1)
    x'  = x + EPS*(J0*gx + conv(gx,J) + inputs + i_norm - x - gy - gy@psi)
    gx  = clip(x - 1, 0, 1),  gy piecewise-linear,  out = mean_t gx_t, max over K

with x0 = 0.01, y0 = 1.  While gx == 0 (everywhere), the system collapses
exactly:
  * y stays exactly 1.0  (y + 0.01*(-1 + 0 + 0 + 1) == y), so gy == 0.21.
  * i_norm == 0.85 (conv of the all-zero s), conv(gx,*) == 0.
  * x_t = a_t * inputs + b_t elementwise, with scalar recurrences
        a_{t+1} = (1-EPS) a_t + EPS,           a_0 = 0
        b_{t+1} = (1-EPS) b_t + EPS*(0.85 - gy - colsum(psi)*gy),  b_0 = 0.01
  * gx_t = clip(a_t*inputs + b_t - 1, 0, 1) stays identically 0 as long as
        max_t (a_t * inputs.max() + b_t) < 1
    which requires inputs.max() >= ~6.66; the model's input domain is [0,1).

Hence out = (1/16) * sum_t clip(a_t*inputs + b_t - 1, 0, 1), and because each
term is nondecreasing in the input value, max over channels commutes with the
whole expression: it is evaluated at m = max_k inputs.

The device kernel computes exactly that: m = channel-max of the input slab
(reads all input bytes - the memory-bound part), then evaluates the sum of
affine-clip terms. Because the clip knots (1-b_t)/a_t decrease with t, for
m < (1-b_15)/a_15 ~= 7.075 the sum equals its t=16 term alone, and that term
stays below 1/16 there, so a single relu-affine evaluates it exactly on the
guard-certified domain.

A host-side guard verifies the collapse precondition (with wide margin) from
the actual inputs/psi and otherwise falls back to a full jax implementation
of the reference on CPU.

Sharding: pure data parallelism, 8 cores x 128 rows of the flattened
(2*512, 512, 12) input.

Device schedule (per core)
--------------------------
Input staged channel-major as 12 planes of [128 rows, 512 cols] bf16.
Four DMA chunks (4+4+2+2 planes) alternate across the two HWDGE rings
(SP + ACT) so transfer runs at the ~380 GB/s aggregate rate; the channel
max is a short balanced tree of wide DVE tensor-tensor maxes (2x mode)
that overlaps the chunk arrivals, one ACT relu-affine produces the
(1/16-folded) output in bf16, and a single DMA writes it back. The host
upcasts to f32. Instruction/semaphore count is kept minimal to shrink
the NEFF preamble/teardown overhead.
"""

import numpy as np

K = 12
STEPS = 16
EPS = 0.01
TX = 1.0
G1 = 0.21
J0 = 0.8
B, H, WD = 2, 512, 512
N_CORES = 8
ROWS = B * H                  # 1024 flattened rows
RPC = ROWS // N_CORES         # 128 rows per core == SBUF partitions
ROWW = WD * K                 # 6144 elements per row

_CACHE = {}


def _coeffs(colsum):
    """Scalar affine recurrence coefficients while gx == 0 (float64)."""
    gy = G1 * 1.0             # y stays exactly 1.0
    drive = 0.85 - gy - colsum * gy
    a, b = 0.0, 0.01
    A, Bc = [], []
    for _ in range(STEPS):
        a = (1.0 - EPS) * a + EPS
        b = (1.0 - EPS) * b + EPS * drive
        A.append(a)
        Bc.append(b)
    return np.array(A), np.array(Bc)


def _build_program(A, Bc):
    import concourse.bacc as bacc
    import concourse.mybir as mybir
    from concourse.tile import TileContext

    bf16 = mybir.dt.bfloat16
    tmax = mybir.AluOpType.max
    tadd = mybir.AluOpType.add

    nc = bacc.Bacc("TRN2", target_bir_lowering=False, debug=False)
    x = nc.dram_tensor("x", [RPC, ROWW], bf16, kind="ExternalInput")
    out = nc.dram_tensor("out", [RPC, WD], bf16, kind="ExternalOutput")

    # relu knot: z = max(m + (b16-1)/a16, 0) is exactly 0 wherever
    # a16*m + b16 < 1, which the host guard certifies for every pixel --
    # so it equals the true output (identically 0) on the whole certified
    # domain without needing the a16/16 rescale (0 * s == 0).
    knot = float((Bc[STEPS - 1] - 1.0) / A[STEPS - 1])

    with TileContext(nc) as tc:
        with tc.tile_pool(name="p", bufs=1) as pool:
            # input chunks: one DMA descriptor per partition-row, and the
            # HW DGE retires ~1 descriptor / 155ns / engine regardless of
            # size -- so 4-plane chunks (4096B segments) are the smallest
            # granule that still sustains the full ~360 GB/s bus rate.
            c1 = pool.tile([RPC, 4 * WD], bf16, tag="c1")
            c2 = pool.tile([RPC, 4 * WD], bf16, tag="c2")
            c3 = pool.tile([RPC, 2 * WD], bf16, tag="c3")
            c4 = pool.tile([RPC, 2 * WD], bf16, tag="c4")
            nc.sync.dma_start(out=c1[:], in_=x[:, 0:2048])
            nc.scalar.dma_start(out=c2[:], in_=x[:, 2048:4096])
            nc.sync.dma_start(out=c3[:], in_=x[:, 4096:5120])
            nc.scalar.dma_start(out=c4[:], in_=x[:, 5120:6144])

            # channel max: wide DVE maxes (bf16 2x mode) ordered to chunk
            # arrival so only the last chunk's fold trails the DMA
            u1 = pool.tile([RPC, 2 * WD], bf16, tag="u1")
            r1 = pool.tile([RPC, WD], bf16, tag="r1")
            u2 = pool.tile([RPC, 2 * WD], bf16, tag="u2")
            r2 = pool.tile([RPC, WD], bf16, tag="r2")
            r12 = pool.tile([RPC, WD], bf16, tag="r12")
            m = pool.tile([RPC, WD], bf16, tag="m")
            z = pool.tile([RPC, WD], bf16, tag="z")

            nc.vector.tensor_tensor(out=u1[:], in0=c1[:, 0:1024],
                                    in1=c1[:, 1024:2048], op=tmax)
            nc.vector.tensor_tensor(out=r1[:], in0=u1[:, 0:512],
                                    in1=u1[:, 512:1024], op=tmax)
            nc.vector.tensor_tensor(out=u2[:], in0=c2[:, 0:1024],
                                    in1=c2[:, 1024:2048], op=tmax)
            nc.vector.tensor_tensor(out=r2[:], in0=u2[:, 0:512],
                                    in1=u2[:, 512:1024], op=tmax)
            nc.vector.tensor_tensor(out=r12[:], in0=r2[:], in1=r1[:], op=tmax)
            # c3 (p8,p9) folds into the running max as soon as it lands;
            # only c4's single pair-max + merge + the affine trail the DMA
            r3 = pool.tile([RPC, WD], bf16, tag="r3")
            r123 = pool.tile([RPC, WD], bf16, tag="r123")
            r4 = pool.tile([RPC, WD], bf16, tag="r4")
            nc.vector.tensor_tensor(out=r3[:], in0=c3[:, 0:512],
                                    in1=c3[:, 512:1024], op=tmax)
            nc.vector.tensor_tensor(out=r123[:], in0=r12[:], in1=r3[:], op=tmax)
            nc.vector.tensor_tensor(out=r4[:], in0=c4[:, 0:512],
                                    in1=c4[:, 512:1024], op=tmax)
            nc.vector.tensor_tensor(out=m[:], in0=r123[:], in1=r4[:], op=tmax)
            nc.vector.tensor_scalar(out=z[:], in0=m[:], scalar1=knot,
                                    scalar2=0.0, op0=tadd, op1=tmax)
            # output: 1024B rows are DGE-descriptor-bound, so split by
            # partition halves across both rings (64 descriptors each)
            nc.sync.dma_start(out=out[0:64, :], in_=z[0:64, :])
            nc.scalar.dma_start(out=out[64:128, :], in_=z[64:128, :])

    nc.compile()
    return nc


def _get_program(A, Bc):
    key = (tuple(np.round(A, 12)), tuple(np.round(Bc, 12)))
    if key not in _CACHE:
        _CACHE[key] = _build_program(A, Bc)
    return _CACHE[key]


def _run_on_device(inputs_np, A, Bc, trace=False):
    from concourse.bass_utils import run_bass_kernel_spmd

    nc = _get_program(A, Bc)
    import ml_dtypes
    flat = np.ascontiguousarray(
        inputs_np.reshape(ROWS, WD, K).transpose(0, 2, 1)
    ).astype(ml_dtypes.bfloat16).reshape(ROWS, ROWW)
    in_maps = [
        {"x": np.ascontiguousarray(flat[i * RPC:(i + 1) * RPC])}
        for i in range(N_CORES)
    ]
    res = run_bass_kernel_spmd(nc, in_maps, list(range(N_CORES)), trace=trace)
    out = np.concatenate([res.results[i]["out"] for i in range(N_CORES)], axis=0)
    return out.reshape(B, H, WD).astype(np.float32), res


def _reference_fallback(inputs, Wk, Jk, psi):
    """Full reference math in jax on CPU (only for out-of-domain inputs)."""
    import jax
    import jax.numpy as jnp

    cpu = jax.devices("cpu")[0]
    with jax.default_device(cpu):
        inputs = jnp.asarray(np.asarray(inputs), jnp.float32)
        Wk = jnp.asarray(np.asarray(Wk), jnp.float32)
        Jk = jnp.asarray(np.asarray(Jk), jnp.float32)
        psi = jnp.asarray(np.asarray(psi), jnp.float32)
        PAD = 7

        def _conv(xx, kk, padding):
            return jax.lax.conv_general_dilated(
                xx, kk, (1, 1), padding,
                dimension_numbers=("NHWC", "HWIO", "NHWC"))

        def _gx(xx):
            return jnp.clip(xx - TX, 0.0, 1.0)

        def _gy(yy):
            yc = jnp.maximum(yy, 0.0)
            return jnp.where(yc <= 1.2, G1 * yc, G1 * 1.2 + 2.5 * (yc - 1.2))

        psi_mat = psi[0, 0]
        box = jnp.ones((5, 5, 1, 1), inputs.dtype)
        x = jnp.full_like(inputs, 0.01)
        y = jnp.ones_like(inputs)
        gx = _gx(x)
        gy = _gy(y)
        out = jnp.zeros_like(inputs)
        for _ in range(STEPS):
            s = jnp.sum(gx, axis=3, keepdims=True)
            i_norm = 0.85 - 2.0 * (_conv(s, box, "SAME") / 25.0) ** 2
            gx_p = jnp.pad(gx, ((0, 0), (PAD, PAD), (PAD, PAD), (0, 0)),
                           mode="symmetric")
            inhib = _conv(gx_p, Wk, "VALID")
            excit = _conv(gx_p, Jk, "VALID")
            inhibs_psi = jnp.einsum("bhwi,io->bhwo", gy, psi_mat)
            y_new = y + EPS * (-y + gx + inhib + 1.0)
            x_inhib = x + gy + inhibs_psi
            x_excit = J0 * gx + excit + inputs + i_norm
            x_new = x + EPS * (x_excit - x_inhib)
            gx = _gx(x_new)
            gy = _gy(y_new)
            x, y = x_new, y_new
            out = out + gx
        out = out / STEPS
        return np.asarray(jnp.max(out, axis=3))


def kernel(inputs, W=None, J=None, psi=None, **_ignored):
    inputs_np = np.asarray(inputs, dtype=np.float32)
    assert inputs_np.shape == (B, H, WD, K), inputs_np.shape

    # Guard: the gx==0 collapse must hold for these inputs/psi.
    ok = True
    colsum = 3.0
    if psi is not None:
        cs = np.asarray(psi, dtype=np.float64)[0, 0].sum(axis=0)
        if np.max(np.abs(cs - cs[0])) < 1e-9:
            colsum = float(cs[0])
        else:
            ok = False
    if ok:
        A, Bc = _coeffs(colsum)
        # 1.004 factor covers bf16 round-up of the staged inputs (<= 2^-8 rel)
        mx = float(inputs_np.max()) * 1.004
        if np.max(A * mx + Bc) >= 0.98:
            ok = False
    if not ok:
        return _reference_fallback(inputs, W, J, psi).astype(np.float32)

    out, _ = _run_on_device(inputs_np, A, Bc)
    return out


if __name__ == "__main__":
    rng = np.random.default_rng(0)
    x = rng.random((B, H, WD, K), dtype=np.float32)
    o = kernel(inputs=x)
    print("kernel out:", o.shape, o.dtype, "maxabs", np.abs(o).max())


# revision 10
# speedup vs baseline: 1.0680x; 1.0680x over previous
"""Trainium2 Bass kernel for nn_BotUpSaliency (B=2, H=W=512, K=12, 16 steps).

Math
----
The reference integrates, for 16 Euler steps (EPS=0.01):

    y'  = y + EPS*(-y + gx + conv(gx,W) + 1)
    x'  = x + EPS*(J0*gx + conv(gx,J) + inputs + i_norm - x - gy - gy@psi)
    gx  = clip(x - 1, 0, 1),  gy piecewise-linear,  out = mean_t gx_t, max over K

with x0 = 0.01, y0 = 1.  While gx == 0 (everywhere), the system collapses
exactly:
  * y stays exactly 1.0  (y + 0.01*(-1 + 0 + 0 + 1) == y), so gy == 0.21.
  * i_norm == 0.85 (conv of the all-zero s), conv(gx,*) == 0.
  * x_t = a_t * inputs + b_t elementwise, with scalar recurrences
        a_{t+1} = (1-EPS) a_t + EPS,           a_0 = 0
        b_{t+1} = (1-EPS) b_t + EPS*(0.85 - gy - colsum(psi)*gy),  b_0 = 0.01
  * gx_t = clip(a_t*inputs + b_t - 1, 0, 1) stays identically 0 as long as
        max_t (a_t * inputs.max() + b_t) < 1
    which requires inputs.max() >= ~6.66; the model's input domain is [0,1).

Hence out = (1/16) * sum_t clip(a_t*inputs + b_t - 1, 0, 1), and because each
term is nondecreasing in the input value, max over channels commutes with the
whole expression: it is evaluated at m = max_k inputs.

The device kernel computes exactly that: m = channel-max of the input slab
(reads all input bytes - the memory-bound part), then evaluates the sum of
affine-clip terms. Because the clip knots (1-b_t)/a_t decrease with t, for
m < (1-b_15)/a_15 ~= 7.075 the sum equals its t=16 term alone, and that term
stays below 1/16 there, so a single relu-affine evaluates it exactly on the
guard-certified domain.

A host-side guard verifies the collapse precondition (with wide margin) from
the actual inputs/psi and otherwise falls back to a full jax implementation
of the reference on CPU.

Sharding: pure data parallelism, 8 cores x 128 rows of the flattened
(2*512, 512, 12) input.

Device schedule (per core)
--------------------------
Input staged channel-major as 12 planes of [128 rows, 512 cols] bf16.
Four DMA chunks (4+4+2+2 planes) alternate across the two HWDGE rings
(SP + ACT) so transfer runs at the ~380 GB/s aggregate rate; the channel
max is a short balanced tree of wide DVE tensor-tensor maxes (2x mode)
that overlaps the chunk arrivals, one ACT relu-affine produces the
(1/16-folded) output in bf16, and a single DMA writes it back. The host
upcasts to f32. Instruction/semaphore count is kept minimal to shrink
the NEFF preamble/teardown overhead.
"""

import numpy as np

K = 12
STEPS = 16
EPS = 0.01
TX = 1.0
G1 = 0.21
J0 = 0.8
B, H, WD = 2, 512, 512
N_CORES = 8
ROWS = B * H                  # 1024 flattened rows
RPC = ROWS // N_CORES         # 128 rows per core == SBUF partitions
ROWW = WD * K                 # 6144 elements per row

_CACHE = {}


def _coeffs(colsum):
    """Scalar affine recurrence coefficients while gx == 0 (float64)."""
    gy = G1 * 1.0             # y stays exactly 1.0
    drive = 0.85 - gy - colsum * gy
    a, b = 0.0, 0.01
    A, Bc = [], []
    for _ in range(STEPS):
        a = (1.0 - EPS) * a + EPS
        b = (1.0 - EPS) * b + EPS * drive
        A.append(a)
        Bc.append(b)
    return np.array(A), np.array(Bc)


def _build_program(A, Bc):
    import concourse.bacc as bacc
    import concourse.mybir as mybir
    from concourse.tile import TileContext

    bf16 = mybir.dt.bfloat16
    tmax = mybir.AluOpType.max
    tadd = mybir.AluOpType.add

    nc = bacc.Bacc("TRN2", target_bir_lowering=False, debug=False)
    x = nc.dram_tensor("x", [RPC, ROWW], bf16, kind="ExternalInput")
    out = nc.dram_tensor("out", [RPC, WD], bf16, kind="ExternalOutput")

    # relu knot: z = max(m + (b16-1)/a16, 0) is exactly 0 wherever
    # a16*m + b16 < 1, which the host guard certifies for every pixel --
    # so it equals the true output (identically 0) on the whole certified
    # domain without needing the a16/16 rescale (0 * s == 0).
    knot = float((Bc[STEPS - 1] - 1.0) / A[STEPS - 1])

    with TileContext(nc) as tc:
        with tc.tile_pool(name="p", bufs=1) as pool:
            # input chunks: one DMA descriptor per partition-row, and the
            # HW DGE retires ~1 descriptor / 155ns / engine regardless of
            # size -- so 4-plane chunks (4096B segments) are the smallest
            # granule that still sustains the full ~360 GB/s bus rate.
            # all input chunks on the SP ring: HWDGE descriptor generation is
            # serialized globally anyway (~12ns/desc), and the second ring's
            # DGE spin-up costs ~1.9us -- one FIFO ring gives back-to-back
            # chunk completions at the same bus-bound total
            c1 = pool.tile([RPC, 4 * WD], bf16, tag="c1")
            c2 = pool.tile([RPC, 4 * WD], bf16, tag="c2")
            c3 = pool.tile([RPC, 4 * WD], bf16, tag="c3")
            nc.sync.dma_start(out=c1[:], in_=x[:, 0:2048])
            nc.sync.dma_start(out=c2[:], in_=x[:, 2048:4096])
            nc.sync.dma_start(out=c3[:], in_=x[:, 4096:6144])

            # channel max: wide DVE maxes (bf16 2x mode) ordered to chunk
            # arrival so only the last chunk's fold trails the DMA
            u1 = pool.tile([RPC, 2 * WD], bf16, tag="u1")
            r1 = pool.tile([RPC, WD], bf16, tag="r1")
            u2 = pool.tile([RPC, 2 * WD], bf16, tag="u2")
            r2 = pool.tile([RPC, WD], bf16, tag="r2")
            r12 = pool.tile([RPC, WD], bf16, tag="r12")
            m = pool.tile([RPC, WD], bf16, tag="m")
            z = pool.tile([RPC, WD], bf16, tag="z")

            nc.vector.tensor_tensor(out=u1[:], in0=c1[:, 0:1024],
                                    in1=c1[:, 1024:2048], op=tmax)
            nc.vector.tensor_tensor(out=r1[:], in0=u1[:, 0:512],
                                    in1=u1[:, 512:1024], op=tmax)
            nc.vector.tensor_tensor(out=u2[:], in0=c2[:, 0:1024],
                                    in1=c2[:, 1024:2048], op=tmax)
            nc.vector.tensor_tensor(out=r2[:], in0=u2[:, 0:512],
                                    in1=u2[:, 512:1024], op=tmax)
            nc.vector.tensor_tensor(out=r12[:], in0=r2[:], in1=r1[:], op=tmax)
            u3 = pool.tile([RPC, 2 * WD], bf16, tag="u3")
            r3 = pool.tile([RPC, WD], bf16, tag="r3")
            nc.vector.tensor_tensor(out=u3[:], in0=c3[:, 0:1024],
                                    in1=c3[:, 1024:2048], op=tmax)
            nc.vector.tensor_tensor(out=r3[:], in0=u3[:, 0:512],
                                    in1=u3[:, 512:1024], op=tmax)
            nc.vector.tensor_tensor(out=m[:], in0=r3[:], in1=r12[:], op=tmax)
            nc.vector.tensor_scalar(out=z[:], in0=m[:], scalar1=knot,
                                    scalar2=0.0, op0=tadd, op1=tmax)
            # output on the otherwise-idle ACT ring
            nc.scalar.dma_start(out=out[:], in_=z[:])

    nc.compile()
    return nc


def _get_program(A, Bc):
    key = (tuple(np.round(A, 12)), tuple(np.round(Bc, 12)))
    if key not in _CACHE:
        _CACHE[key] = _build_program(A, Bc)
    return _CACHE[key]


def _run_on_device(inputs_np, A, Bc, trace=False):
    from concourse.bass_utils import run_bass_kernel_spmd

    nc = _get_program(A, Bc)
    import ml_dtypes
    flat = np.ascontiguousarray(
        inputs_np.reshape(ROWS, WD, K).transpose(0, 2, 1)
    ).astype(ml_dtypes.bfloat16).reshape(ROWS, ROWW)
    in_maps = [
        {"x": np.ascontiguousarray(flat[i * RPC:(i + 1) * RPC])}
        for i in range(N_CORES)
    ]
    res = run_bass_kernel_spmd(nc, in_maps, list(range(N_CORES)), trace=trace)
    out = np.concatenate([res.results[i]["out"] for i in range(N_CORES)], axis=0)
    return out.reshape(B, H, WD).astype(np.float32), res


def _reference_fallback(inputs, Wk, Jk, psi):
    """Full reference math in jax on CPU (only for out-of-domain inputs)."""
    import jax
    import jax.numpy as jnp

    cpu = jax.devices("cpu")[0]
    with jax.default_device(cpu):
        inputs = jnp.asarray(np.asarray(inputs), jnp.float32)
        Wk = jnp.asarray(np.asarray(Wk), jnp.float32)
        Jk = jnp.asarray(np.asarray(Jk), jnp.float32)
        psi = jnp.asarray(np.asarray(psi), jnp.float32)
        PAD = 7

        def _conv(xx, kk, padding):
            return jax.lax.conv_general_dilated(
                xx, kk, (1, 1), padding,
                dimension_numbers=("NHWC", "HWIO", "NHWC"))

        def _gx(xx):
            return jnp.clip(xx - TX, 0.0, 1.0)

        def _gy(yy):
            yc = jnp.maximum(yy, 0.0)
            return jnp.where(yc <= 1.2, G1 * yc, G1 * 1.2 + 2.5 * (yc - 1.2))

        psi_mat = psi[0, 0]
        box = jnp.ones((5, 5, 1, 1), inputs.dtype)
        x = jnp.full_like(inputs, 0.01)
        y = jnp.ones_like(inputs)
        gx = _gx(x)
        gy = _gy(y)
        out = jnp.zeros_like(inputs)
        for _ in range(STEPS):
            s = jnp.sum(gx, axis=3, keepdims=True)
            i_norm = 0.85 - 2.0 * (_conv(s, box, "SAME") / 25.0) ** 2
            gx_p = jnp.pad(gx, ((0, 0), (PAD, PAD), (PAD, PAD), (0, 0)),
                           mode="symmetric")
            inhib = _conv(gx_p, Wk, "VALID")
            excit = _conv(gx_p, Jk, "VALID")
            inhibs_psi = jnp.einsum("bhwi,io->bhwo", gy, psi_mat)
            y_new = y + EPS * (-y + gx + inhib + 1.0)
            x_inhib = x + gy + inhibs_psi
            x_excit = J0 * gx + excit + inputs + i_norm
            x_new = x + EPS * (x_excit - x_inhib)
            gx = _gx(x_new)
            gy = _gy(y_new)
            x, y = x_new, y_new
            out = out + gx
        out = out / STEPS
        return np.asarray(jnp.max(out, axis=3))


def kernel(inputs, W=None, J=None, psi=None, **_ignored):
    inputs_np = np.asarray(inputs, dtype=np.float32)
    assert inputs_np.shape == (B, H, WD, K), inputs_np.shape

    # Guard: the gx==0 collapse must hold for these inputs/psi.
    ok = True
    colsum = 3.0
    if psi is not None:
        cs = np.asarray(psi, dtype=np.float64)[0, 0].sum(axis=0)
        if np.max(np.abs(cs - cs[0])) < 1e-9:
            colsum = float(cs[0])
        else:
            ok = False
    if ok:
        A, Bc = _coeffs(colsum)
        # 1.004 factor covers bf16 round-up of the staged inputs (<= 2^-8 rel)
        mx = float(inputs_np.max()) * 1.004
        if np.max(A * mx + Bc) >= 0.98:
            ok = False
    if not ok:
        return _reference_fallback(inputs, W, J, psi).astype(np.float32)

    out, _ = _run_on_device(inputs_np, A, Bc)
    return out


if __name__ == "__main__":
    rng = np.random.default_rng(0)
    x = rng.random((B, H, WD, K), dtype=np.float32)
    o = kernel(inputs=x)
    print("kernel out:", o.shape, o.dtype, "maxabs", np.abs(o).max())


# revision 12
# speedup vs baseline: 1.0788x; 1.0101x over previous
"""Trainium2 Bass kernel for nn_BotUpSaliency (B=2, H=W=512, K=12, 16 steps).

Math
----
The reference integrates, for 16 Euler steps (EPS=0.01):

    y'  = y + EPS*(-y + gx + conv(gx,W) + 1)
    x'  = x + EPS*(J0*gx + conv(gx,J) + inputs + i_norm - x - gy - gy@psi)
    gx  = clip(x - 1, 0, 1),  gy piecewise-linear,  out = mean_t gx_t, max over K

with x0 = 0.01, y0 = 1.  While gx == 0 (everywhere), the system collapses
exactly:
  * y stays exactly 1.0  (y + 0.01*(-1 + 0 + 0 + 1) == y), so gy == 0.21.
  * i_norm == 0.85 (conv of the all-zero s), conv(gx,*) == 0.
  * x_t = a_t * inputs + b_t elementwise, with scalar recurrences
        a_{t+1} = (1-EPS) a_t + EPS,           a_0 = 0
        b_{t+1} = (1-EPS) b_t + EPS*(0.85 - gy - colsum(psi)*gy),  b_0 = 0.01
  * gx_t = clip(a_t*inputs + b_t - 1, 0, 1) stays identically 0 as long as
        max_t (a_t * inputs.max() + b_t) < 1
    which requires inputs.max() >= ~6.66; the model's input domain is [0,1).

Hence out = (1/16) * sum_t clip(a_t*inputs + b_t - 1, 0, 1), and because each
term is nondecreasing in the input value, max over channels commutes with the
whole expression: it is evaluated at m = max_k inputs.

The device kernel computes exactly that: m = channel-max of the input slab
(reads all input bytes - the memory-bound part), then evaluates the sum of
affine-clip terms. Because the clip knots (1-b_t)/a_t decrease with t, for
m < (1-b_15)/a_15 ~= 7.075 the sum equals its t=16 term alone, and that term
stays below 1/16 there, so a single relu-affine evaluates it exactly on the
guard-certified domain.

A host-side guard verifies the collapse precondition (with wide margin) from
the actual inputs/psi and otherwise falls back to a full jax implementation
of the reference on CPU.

Sharding: pure data parallelism, 8 cores x 128 rows of the flattened
(2*512, 512, 12) input.

Device schedule (per core)
--------------------------
Input staged channel-major as 12 planes of [128 rows, 512 cols] bf16.
Four DMA chunks (4+4+2+2 planes) alternate across the two HWDGE rings
(SP + ACT) so transfer runs at the ~380 GB/s aggregate rate; the channel
max is a short balanced tree of wide DVE tensor-tensor maxes (2x mode)
that overlaps the chunk arrivals, one ACT relu-affine produces the
(1/16-folded) output in bf16, and a single DMA writes it back. The host
upcasts to f32. Instruction/semaphore count is kept minimal to shrink
the NEFF preamble/teardown overhead.
"""

import numpy as np

K = 12
STEPS = 16
EPS = 0.01
TX = 1.0
G1 = 0.21
J0 = 0.8
B, H, WD = 2, 512, 512
N_CORES = 8
ROWS = B * H                  # 1024 flattened rows
RPC = ROWS // N_CORES         # 128 rows per core == SBUF partitions
ROWW = WD * K                 # 6144 elements per row

_CACHE = {}


def _coeffs(colsum):
    """Scalar affine recurrence coefficients while gx == 0 (float64)."""
    gy = G1 * 1.0             # y stays exactly 1.0
    drive = 0.85 - gy - colsum * gy
    a, b = 0.0, 0.01
    A, Bc = [], []
    for _ in range(STEPS):
        a = (1.0 - EPS) * a + EPS
        b = (1.0 - EPS) * b + EPS * drive
        A.append(a)
        Bc.append(b)
    return np.array(A), np.array(Bc)


def _build_program(A, Bc):
    import concourse.bacc as bacc
    import concourse.mybir as mybir
    from concourse.tile import TileContext

    bf16 = mybir.dt.bfloat16
    tmax = mybir.AluOpType.max
    tadd = mybir.AluOpType.add

    nc = bacc.Bacc("TRN2", target_bir_lowering=False, debug=False)
    x = nc.dram_tensor("x", [RPC, ROWW], bf16, kind="ExternalInput")
    out = nc.dram_tensor("out", [RPC, WD], bf16, kind="ExternalOutput")

    # relu knot: z = max(m + (b16-1)/a16, 0) is exactly 0 wherever
    # a16*m + b16 < 1, which the host guard certifies for every pixel --
    # so it equals the true output (identically 0) on the whole certified
    # domain without needing the a16/16 rescale (0 * s == 0).
    knot = float((Bc[STEPS - 1] - 1.0) / A[STEPS - 1])

    with TileContext(nc) as tc:
        with tc.tile_pool(name="p", bufs=1) as pool:
            # input chunks: one DMA descriptor per partition-row, and the
            # HW DGE retires ~1 descriptor / 155ns / engine regardless of
            # size -- so 4-plane chunks (4096B segments) are the smallest
            # granule that still sustains the full ~360 GB/s bus rate.
            # all input chunks on the SP ring: HWDGE descriptor generation is
            # serialized globally anyway (~12ns/desc), and the second ring's
            # DGE spin-up costs ~1.9us -- one FIFO ring gives back-to-back
            # chunk completions at the same bus-bound total
            c1 = pool.tile([RPC, 4 * WD], bf16, tag="c1")
            c2 = pool.tile([RPC, 4 * WD], bf16, tag="c2")
            c3 = pool.tile([RPC, 2 * WD], bf16, tag="c3")
            c4 = pool.tile([RPC, 2 * WD], bf16, tag="c4")
            nc.sync.dma_start(out=c1[:], in_=x[:, 0:2048])
            nc.sync.dma_start(out=c2[:], in_=x[:, 2048:4096])
            nc.sync.dma_start(out=c3[:], in_=x[:, 4096:5120])
            nc.sync.dma_start(out=c4[:], in_=x[:, 5120:6144])

            # channel max: wide DVE maxes (bf16 2x mode) ordered to chunk
            # arrival so only the last chunk's fold trails the DMA
            u1 = pool.tile([RPC, 2 * WD], bf16, tag="u1")
            r1 = pool.tile([RPC, WD], bf16, tag="r1")
            u2 = pool.tile([RPC, 2 * WD], bf16, tag="u2")
            r2 = pool.tile([RPC, WD], bf16, tag="r2")
            r12 = pool.tile([RPC, WD], bf16, tag="r12")
            m = pool.tile([RPC, WD], bf16, tag="m")
            z = pool.tile([RPC, WD], bf16, tag="z")

            nc.vector.tensor_tensor(out=u1[:], in0=c1[:, 0:1024],
                                    in1=c1[:, 1024:2048], op=tmax)
            nc.vector.tensor_tensor(out=r1[:], in0=u1[:, 0:512],
                                    in1=u1[:, 512:1024], op=tmax)
            nc.vector.tensor_tensor(out=u2[:], in0=c2[:, 0:1024],
                                    in1=c2[:, 1024:2048], op=tmax)
            nc.vector.tensor_tensor(out=r2[:], in0=u2[:, 0:512],
                                    in1=u2[:, 512:1024], op=tmax)
            nc.vector.tensor_tensor(out=r12[:], in0=r2[:], in1=r1[:], op=tmax)
            # c3 folds into the running max while c4 is still in flight, so
            # only r4 + the final merge + the affine trail the DMA
            r3 = pool.tile([RPC, WD], bf16, tag="r3")
            r123 = pool.tile([RPC, WD], bf16, tag="r123")
            r4 = pool.tile([RPC, WD], bf16, tag="r4")
            nc.vector.tensor_tensor(out=r3[:], in0=c3[:, 0:512],
                                    in1=c3[:, 512:1024], op=tmax)
            nc.vector.tensor_tensor(out=r123[:], in0=r12[:], in1=r3[:], op=tmax)
            nc.vector.tensor_tensor(out=r4[:], in0=c4[:, 0:512],
                                    in1=c4[:, 512:1024], op=tmax)
            nc.vector.tensor_tensor(out=m[:], in0=r123[:], in1=r4[:], op=tmax)
            nc.vector.tensor_scalar(out=z[:], in0=m[:], scalar1=knot,
                                    scalar2=0.0, op0=tadd, op1=tmax)
            # output on the otherwise-idle ACT ring
            nc.scalar.dma_start(out=out[:], in_=z[:])

    nc.compile()
    return nc


def _get_program(A, Bc):
    key = (tuple(np.round(A, 12)), tuple(np.round(Bc, 12)))
    if key not in _CACHE:
        _CACHE[key] = _build_program(A, Bc)
    return _CACHE[key]


def _run_on_device(inputs_np, A, Bc, trace=False):
    from concourse.bass_utils import run_bass_kernel_spmd

    nc = _get_program(A, Bc)
    import ml_dtypes
    flat = np.ascontiguousarray(
        inputs_np.reshape(ROWS, WD, K).transpose(0, 2, 1)
    ).astype(ml_dtypes.bfloat16).reshape(ROWS, ROWW)
    in_maps = [
        {"x": np.ascontiguousarray(flat[i * RPC:(i + 1) * RPC])}
        for i in range(N_CORES)
    ]
    res = run_bass_kernel_spmd(nc, in_maps, list(range(N_CORES)), trace=trace)
    out = np.concatenate([res.results[i]["out"] for i in range(N_CORES)], axis=0)
    return out.reshape(B, H, WD).astype(np.float32), res


def _reference_fallback(inputs, Wk, Jk, psi):
    """Full reference math in jax on CPU (only for out-of-domain inputs)."""
    import jax
    import jax.numpy as jnp

    cpu = jax.devices("cpu")[0]
    with jax.default_device(cpu):
        inputs = jnp.asarray(np.asarray(inputs), jnp.float32)
        Wk = jnp.asarray(np.asarray(Wk), jnp.float32)
        Jk = jnp.asarray(np.asarray(Jk), jnp.float32)
        psi = jnp.asarray(np.asarray(psi), jnp.float32)
        PAD = 7

        def _conv(xx, kk, padding):
            return jax.lax.conv_general_dilated(
                xx, kk, (1, 1), padding,
                dimension_numbers=("NHWC", "HWIO", "NHWC"))

        def _gx(xx):
            return jnp.clip(xx - TX, 0.0, 1.0)

        def _gy(yy):
            yc = jnp.maximum(yy, 0.0)
            return jnp.where(yc <= 1.2, G1 * yc, G1 * 1.2 + 2.5 * (yc - 1.2))

        psi_mat = psi[0, 0]
        box = jnp.ones((5, 5, 1, 1), inputs.dtype)
        x = jnp.full_like(inputs, 0.01)
        y = jnp.ones_like(inputs)
        gx = _gx(x)
        gy = _gy(y)
        out = jnp.zeros_like(inputs)
        for _ in range(STEPS):
            s = jnp.sum(gx, axis=3, keepdims=True)
            i_norm = 0.85 - 2.0 * (_conv(s, box, "SAME") / 25.0) ** 2
            gx_p = jnp.pad(gx, ((0, 0), (PAD, PAD), (PAD, PAD), (0, 0)),
                           mode="symmetric")
            inhib = _conv(gx_p, Wk, "VALID")
            excit = _conv(gx_p, Jk, "VALID")
            inhibs_psi = jnp.einsum("bhwi,io->bhwo", gy, psi_mat)
            y_new = y + EPS * (-y + gx + inhib + 1.0)
            x_inhib = x + gy + inhibs_psi
            x_excit = J0 * gx + excit + inputs + i_norm
            x_new = x + EPS * (x_excit - x_inhib)
            gx = _gx(x_new)
            gy = _gy(y_new)
            x, y = x_new, y_new
            out = out + gx
        out = out / STEPS
        return np.asarray(jnp.max(out, axis=3))


def kernel(inputs, W=None, J=None, psi=None, **_ignored):
    inputs_np = np.asarray(inputs, dtype=np.float32)
    assert inputs_np.shape == (B, H, WD, K), inputs_np.shape

    # Guard: the gx==0 collapse must hold for these inputs/psi.
    ok = True
    colsum = 3.0
    if psi is not None:
        cs = np.asarray(psi, dtype=np.float64)[0, 0].sum(axis=0)
        if np.max(np.abs(cs - cs[0])) < 1e-9:
            colsum = float(cs[0])
        else:
            ok = False
    if ok:
        A, Bc = _coeffs(colsum)
        # 1.004 factor covers bf16 round-up of the staged inputs (<= 2^-8 rel)
        mx = float(inputs_np.max()) * 1.004
        if np.max(A * mx + Bc) >= 0.98:
            ok = False
    if not ok:
        return _reference_fallback(inputs, W, J, psi).astype(np.float32)

    out, _ = _run_on_device(inputs_np, A, Bc)
    return out


if __name__ == "__main__":
    rng = np.random.default_rng(0)
    x = rng.random((B, H, WD, K), dtype=np.float32)
    o = kernel(inputs=x)
    print("kernel out:", o.shape, o.dtype, "maxabs", np.abs(o).max())
